# revision 1
# baseline (speedup 1.0000x reference)
"""Trainium2 Bass kernel for nn_BandSplit.

Computes, for each of K mel bands:
    out[b, o, t, k] = sum_{c,w} x[b, c, t, idx[k,w]] * mel_w[k,w] * pre_w[k,c,w,o] + pre_b[k,o]

Structure exploited:
  - Band indices idx[k, :n_k] are contiguous runs (triangular mel filters),
    so the gather is a strided slice.
  - mel_w folds into pre_w on the host: W2[k,c,w,o] = mel_w[k,w]*pre_w[k,c,w,o].
  - With x rows laid out channel-interleaved (row = 2f + c), band k's whole
    contraction (both channels) is the contiguous row run [2s_k, 2s_k+2n_k).
    Each band is then 1-3 matmuls (chunk-boundary splits): contraction over
    those rows, free dims O=128 x (B*T_loc) columns, accumulated in PSUM.
  - The tensor engine requires operand base partitions to be 32-aligned
    (tile_position rule).  Pieces are extended DOWN to an aligned base with
    zero weight rows — zero extra x bytes, a few zero rows in the packed
    weights.

Sharding: data-parallel over T across 8 cores (T=1024 -> 128/core); identical
SPMD program per core, weights replicated, host reassembles (B, O, T, K).

Data movement: everything is staged partition-major so each DMA is a large
[128, N] <- [128, N] transfer with per-partition-contiguous source (~128
descriptors).  x loads once (no per-band gather DMAs — v1 was bound by the
Sync sequencer's ~930 ns/DMA descriptor generation).  Inputs are cast to
bf16 on the host (PSUM accumulation stays fp32; output fp32): halves DMA
traffic and keeps matmuls single-pass (fp32 matmuls lower to two PE passes).
Input loads issue on the Sync (SP) HWDGE ring; output DMAs issue on the
GpSimd (SWDGE) ring so neither queues behind the other, and the compute
engines (PE matmuls, DVE/ACT PSUM->SBUF copies) never issue DMAs.
PE warm-up matmuls run during the preload so the HAM clock gate lifts the
PE from 1.2 to 2.4 GHz before the real stream starts.
"""

import os
import sys
import types

import numpy as np

for _p in ("/opt/trn_rl_repo",):
    if _p not in sys.path:
        sys.path.insert(0, _p)

import ml_dtypes

import concourse.bass as bass
import concourse.mybir as mybir
import concourse.tile as tile
from concourse import bass_utils

N_CORES = 8
O = 128          # out channels (= stationary free dim = PSUM partitions)
GROUP = 4        # bands per weight-packing unit / compute group
P = 128          # SBUF partitions / chunk rows
BT = 512         # B * T_loc columns per core
N_WARMUP = int(os.environ.get("BANDSPLIT_WARMUP", "28"))

_F32 = mybir.dt.float32

if os.environ.get("BANDSPLIT_DTYPE", "bf16") == "f32":
    _IN_DT = mybir.dt.float32
    _IN_NP = np.float32
else:
    _IN_DT = mybir.dt.bfloat16
    _IN_NP = ml_dtypes.bfloat16


# ---------------------------------------------------------------------------
# Workaround: this container's walrus rejects instructions carrying more than
# a couple of sem waits ("Too many sync wait commands", CoreV3GenImpl
# setupSyncWait).  Post-pass: move excess waits onto single-wait NoOps
# inserted just before the instruction on the same engine/sequencer.
# ---------------------------------------------------------------------------
_MAX_WAITS = 1

if os.environ.get("BANDSPLIT_LDWOPT"):
    # Experiment: let walrus overlap LDWEIGHTS with matmuls (the default
    # pipeline pins --enable-ldw-opt=false).
    _orig_run_command = bass_utils.run_command

    def _patched_run_command(cmd, **kw):
        if isinstance(cmd, list):
            cmd = [
                "--enable-ldw-opt=true" if c == "--enable-ldw-opt=false" else c
                for c in cmd
            ]
        return _orig_run_command(cmd, **kw)

    bass_utils.run_command = _patched_run_command


def _split_excess_waits(nc, max_waits=_MAX_WAITS):
    ctr = 0
    for f in nc.m.functions:
        for bb in f.blocks:
            il = bb.instructions
            i = 0
            while i < len(il):
                inst = il[i]
                si = inst.sync_info
                if si is not None and si.on_wait and len(si.on_wait) > max_waits:
                    waits = list(si.on_wait)
                    keep = waits[-max_waits:]
                    extra = waits[:-max_waits]
                    nops = []
                    for w in extra:
                        ctr += 1
                        nop = mybir.InstNoOp(
                            name=f"{inst.name}-wsplit{ctr}",
                            engine=inst.engine,
                            sync_info=mybir.SyncInfo(on_wait=[w], on_update=[]),
                            bass_nofuse=True,
                        )
                        nc.register_instruction(nop, overwrite=True)
                        nops.append(nop)
                    inst.sync_info = mybir.SyncInfo(
                        on_wait=keep, on_update=list(si.on_update or [])
                    )
                    il[i:i] = nops
                    i += len(nops)
                i += 1
    return ctr


# ---------------------------------------------------------------------------
# Optional NTFF profiling (test.py sets BANDSPLIT_TRACE=1).  The agent image's
# antenv lacks axon_hooks, so tracing degrades silently unless we install the
# ctypes-based hook ourselves.
# ---------------------------------------------------------------------------
def _install_trace_hook():
    try:
        import antenv  # noqa: F401
        from trn_agent_boot.trn_boot import _ntff_profile_via_ctypes

        if "antenv.axon_hooks" in sys.modules:
            return True
        hook = _ntff_profile_via_ctypes("/opt/axon/libaxon_pjrt.so")
        mod = types.ModuleType("antenv.axon_hooks")
        mod._hook = hook
        mod.get_axon_ntff_profile_hook = lambda: mod._hook
        mod.set_axon_ntff_profile_hook = lambda h: setattr(mod, "_hook", h)
        sys.modules["antenv.axon_hooks"] = mod
        import antenv as _ae

        _ae.axon_hooks = mod
        return True
    except Exception:
        return False


# ---------------------------------------------------------------------------
# Band structure extraction (host side, from the actual inputs)
# ---------------------------------------------------------------------------
def _band_structure(idx, mel_w):
    idx = np.asarray(idx)
    mel_w = np.asarray(mel_w)
    K = idx.shape[0]
    starts = np.empty(K, dtype=np.int64)
    lengths = np.empty(K, dtype=np.int64)
    for k in range(K):
        nz = np.nonzero(mel_w[k])[0]
        assert nz.size > 0, f"band {k} empty"
        n = int(nz.max()) + 1
        run = idx[k, :n]
        assert np.all(np.diff(run) == 1), f"band {k} indices not contiguous"
        starts[k] = int(run[0])
        lengths[k] = n
    return starts, lengths


_FORCE_BASE0 = bool(os.environ.get("BANDSPLIT_BASE0"))


def _align_base(p0, e):
    """Largest legal 32-aligned base <= p0 for a piece ending at e.

    tile_position rule: rows<=32 -> base in {0,32,64,96}; rows<=64 -> {0,64};
    rows>64 -> base 0.
    """
    if _FORCE_BASE0:
        return 0
    for a in (96, 64, 32, 0):
        if a > p0:
            continue
        rows = e - a
        if rows <= 32 or (rows <= 64 and a in (0, 64)) or a == 0:
            return a
    raise AssertionError((p0, e))


# HW note: nonzero tile_position row bases are only safe for single-matmul
# bands (start=stop=True).  Mixing bases inside a PSUM accumulation group
# (split bands) aborts the NEFF at runtime on this stack — v2 of this kernel
# only ever used nonzero bases on single-piece bands and ran fine; enabling
# them on split bands reproducibly failed.  So: split bands go to base 0.


def _plan(starts, lengths, F):
    """Plan pieces and the packed weight-column layout.

    Weight columns are packed per WPACK-band block so each block's columns
    form a contiguous range (one weight DMA per block, prefetchable).
    WPACK=2*GROUP keeps fragmentation low (bigger bins pack tighter than
    per-group bins) while still allowing early prefetch.

    Returns:
      pieces[k]   -> list of (chunk, base, p0, e, wcol)  [rows = e-base incl.
                     zero-extension [base,p0); real weight rows at [p0,e)]
      n_xch       -> number of 128-row x chunks (ceil(2F/128))
      n_wcol      -> number of packed 128-row weight columns
      wseg_ranges -> per WPACK block: (first_col, last_col) inclusive
    """
    K = len(starts)
    n_xch = (2 * F + P - 1) // P
    WPACK = 2 * GROUP

    pieces = [[] for _ in range(K)]
    col_fill = []  # per column: next free 32-slot index (0..4)
    wseg_ranges = []
    col_base = 0

    for k in range(K):
        if k % WPACK == 0:
            if k:
                wseg_ranges.append((col_base, col_base + len(col_fill) - 1))
                col_base += len(col_fill)
            col_fill = []
        r0 = 2 * int(starts[k])
        r1 = r0 + 2 * int(lengths[k])
        single_piece = (r0 % P) + (r1 - r0) <= P
        r = r0
        while r < r1:
            chunk = r // P
            p0 = r % P
            e = min(r1 - chunk * P, P)
            a = _align_base(p0, e) if single_piece else 0
            # place [a, e) into a weight column (32-granule disjointness)
            s_lo, s_hi = a // 32, (e + 31) // 32
            wcol = None
            for ci, fill in enumerate(col_fill):
                if fill <= s_lo:
                    wcol = ci
                    col_fill[ci] = s_hi
                    break
            if wcol is None:
                col_fill.append(s_hi)
                wcol = len(col_fill) - 1
            pieces[k].append((chunk, a, p0, e, col_base + wcol))
            r = chunk * P + e
    wseg_ranges.append((col_base, col_base + len(col_fill) - 1))
    n_wcol = col_base + len(col_fill)
    return pieces, n_xch, n_wcol, wseg_ranges


# ---------------------------------------------------------------------------
# Device program
# ---------------------------------------------------------------------------
def _build_program(pieces, n_xch, n_wcol, wseg_ranges, K, with_bias, split_waits=True):
    nc = bass.Bass("TRN2", target_bir_lowering=False, debug=False)
    xg = nc.dram_tensor("xg", [P, n_xch * BT], _IN_DT, kind="ExternalInput").ap()
    wg = nc.dram_tensor("wg", [P, n_wcol * O], _IN_DT, kind="ExternalInput").ap()
    bt = nc.dram_tensor("bt", [O, K], _F32, kind="ExternalInput").ap()
    out = nc.dram_tensor("out", [O, K * BT], _F32, kind="ExternalOutput").ap()

    n_groups = K // GROUP
    # Process groups with the most matmul work FIRST (the high mel bands).
    # While the output-DMA stream ramps up, the PE has dense work and never
    # idles long enough for the HAM clock gate to re-throttle it to 1.2 GHz;
    # the cheap groups run at the end, where the output stream alone is the
    # constraint and PE stalls are harmless.
    work = [sum(e - a for k in range(g * GROUP, (g + 1) * GROUP)
                for (_, a, _, e, _) in pieces[k]) +
            64 * sum(len(pieces[k]) for k in range(g * GROUP, (g + 1) * GROUP))
            for g in range(n_groups)]
    gorder = list(range(n_groups))  # natural order: loads front-load before outs

    gclo = [min(c for k in range(g * GROUP, (g + 1) * GROUP) for (c, _, _, _, _) in pieces[k])
            for g in range(n_groups)]
    gchi = [max(c for k in range(g * GROUP, (g + 1) * GROUP) for (c, _, _, _, _) in pieces[k])
            for g in range(n_groups)]

    # x segments: walk groups in processing order, each segment covering the
    # not-yet-loaded chunk range the next few groups need.  4 segments.
    seg_of_pos = []
    segs = []
    covered = set()
    per_seg = (n_groups + 3) // 4
    for si in range(4):
        gs = gorder[si * per_seg : (si + 1) * per_seg]
        if not gs:
            continue
        need = set()
        for g in gs:
            need.update(range(gclo[g], gchi[g] + 1))
        need -= covered
        if need:
            segs.append((min(need), max(need)))
            covered.update(range(min(need), max(need) + 1))
        seg_of_pos.append(len(segs) - 1)
    # any chunks never referenced (padding) are irrelevant
    chunk_seg = {}
    for si, (clo, chi) in enumerate(segs):
        for c in range(clo, chi + 1):
            chunk_seg.setdefault(c, si)

    import contextlib

    with tile.TileContext(nc) as tc:
        with contextlib.ExitStack() as ctx:
            stage_pool = ctx.enter_context(tc.tile_pool(name="stage", bufs=6))
            psum_pool = ctx.enter_context(
                tc.tile_pool(name="psum", bufs=4, space="PSUM")
            )
            bias_pool = ctx.enter_context(tc.tile_pool(name="bias", bufs=1))
            warm_pool = ctx.enter_context(tc.tile_pool(name="warm", bufs=1))

            # --- PE warm-up: the HAM clock gate keeps the PE at 1.2 GHz
            # (K=4/8) until it has seen ~3.4us of sustained matmul activity.
            # Burn dummy matmuls during the DMA preload (PE is idle anyway)
            # so the real stream runs at 2.4 GHz.  Measured without this:
            # the PE stayed cold essentially the whole kernel.
            if N_WARMUP:
                wdum = warm_pool.tile([P, O + BT], _IN_DT)
                nc.vector.memset(wdum[:, :], 0)
                for _ in range(N_WARMUP):
                    pw = psum_pool.tile([O, 2 * BT], _F32, tag="psum")
                    nc.tensor.matmul(
                        pw[:, :BT],
                        wdum[:, :O],
                        wdum[:, O : O + BT],
                        start=True,
                        stop=True,
                        tile_position=(0, 0),
                    )

            bias_t = bias_pool.tile([O, K], _F32)
            nc.sync.dma_start(out=bias_t[:, :], in_=bt[:, :])

            # input loads on the Sync ring, in consumption (processing)
            # order: each x segment followed by the weight segments of the
            # groups in that position block.
            xtiles = [None] * len(segs)
            wtiles = [None] * n_groups
            emitted_seg = set()
            for si in range(4):
                gs = gorder[si * per_seg : (si + 1) * per_seg]
                if not gs:
                    continue
                sidx = seg_of_pos[si]
                if sidx >= 0 and sidx not in emitted_seg:
                    emitted_seg.add(sidx)
                    clo, chi = segs[sidx]
                    span = chi - clo + 1
                    xp = ctx.enter_context(
                        tc.tile_pool(name=f"xseg{sidx}", bufs=1)
                    )
                    xt_s = xp.tile([P, span * BT], _IN_DT)
                    nc.sync.dma_start(
                        out=xt_s[:, :], in_=xg[:, clo * BT : (chi + 1) * BT]
                    )
                    xtiles[sidx] = (xt_s, clo)
                for g in gs:
                    wb = g // 2  # weight block = 2 groups (WPACK bands)
                    if wtiles[wb] is not None:
                        continue
                    wlo, whi = wseg_ranges[wb]
                    wp = ctx.enter_context(tc.tile_pool(name=f"wseg{wb}", bufs=1))
                    wt_b = wp.tile([P, (whi - wlo + 1) * O], _IN_DT)
                    nc.sync.dma_start(
                        out=wt_b[:, :], in_=wg[:, wlo * O : (whi + 1) * O]
                    )
                    wtiles[wb] = (wt_b, wlo)

            for g in gorder:
                stage = stage_pool.tile([O, GROUP * BT], _F32, tag="stage")
                wt_g, wlo = wtiles[g // 2]
                # Bands processed in pairs sharing a 2-bank PSUM tile: halves
                # the psum-slot semaphore checks on the PE (micro-idles there
                # are what re-throttle the HAM clock gate) and halves the
                # PSUM->SBUF copy op count.
                for jp in range(GROUP // 2):
                    psum = psum_pool.tile([O, 2 * BT], _F32, tag="psum")
                    for jj in range(2):
                        j = jp * 2 + jj
                        k = g * GROUP + j
                        plist = pieces[k]
                        pslice = psum[:, jj * BT : (jj + 1) * BT]
                        for pi, (c, a, p0, e, wcol) in enumerate(plist):
                            xt_s, clo = xtiles[chunk_seg[c]]
                            lc = c - clo
                            wc = wcol - wlo
                            nc.tensor.matmul(
                                pslice,
                                wt_g[a:e, wc * O : (wc + 1) * O],
                                xt_s[a:e, lc * BT : (lc + 1) * BT],
                                start=(pi == 0),
                                stop=(pi == len(plist) - 1),
                                tile_position=(a, 0),
                            )
                    if with_bias:
                        # bias-fused copies stay per-band (per-partition
                        # scalar differs per band); DVE-only (the ACT
                        # Identity+bias path hit a runtime failure here)
                        for jj in range(2):
                            j = jp * 2 + jj
                            k = g * GROUP + j
                            nc.vector.tensor_scalar_add(
                                out=stage[:, j * BT : (j + 1) * BT],
                                in0=psum[:, jj * BT : (jj + 1) * BT],
                                scalar1=bias_t[:, k : k + 1],
                            )
                    else:
                        dst = stage[:, jp * 2 * BT : (jp * 2 + 2) * BT]
                        if jp % 2 == 0:
                            nc.vector.tensor_copy(dst, psum[:, :])
                        else:
                            nc.scalar.copy(dst, psum[:, :])
                # GpSimd/SWDGE ring: keeps outputs off the Sync ring and
                # off the compute engines.
                nc.gpsimd.dma_start(
                    out=out[:, g * GROUP * BT : (g + 1) * GROUP * BT],
                    in_=stage[:, :],
                )
    if split_waits:
        _split_excess_waits(nc)
    return nc


_CACHE = {}
LAST_RESULTS = None


def kernel(x, idx, mel_w, pre_w, pre_b):
    global LAST_RESULTS
    x = np.ascontiguousarray(np.asarray(x, dtype=np.float32))
    pre_w = np.asarray(pre_w, dtype=np.float32)
    pre_b = np.asarray(pre_b, dtype=np.float32)
    mel_w = np.asarray(mel_w, dtype=np.float32)
    B, C, T, F = x.shape
    K = np.asarray(idx).shape[0]
    assert C == 2 and T % N_CORES == 0
    T_loc = T // N_CORES
    assert B * T_loc == BT and pre_w.shape[-1] == O and K % GROUP == 0

    starts, lengths = _band_structure(idx, mel_w)
    with_bias = bool(np.any(pre_b != 0.0))
    key = (B, C, T, F, K, with_bias, starts.tobytes(), lengths.tobytes())
    if key not in _CACHE:
        pieces, n_xch, n_wcol, wseg_ranges = _plan(starts, lengths, F)
        nc = _build_program(pieces, n_xch, n_wcol, wseg_ranges, K, with_bias)
        _CACHE[key] = (nc, pieces, n_xch, n_wcol)
    nc, pieces, n_xch, n_wcol = _CACHE[key]

    # ---- weights: fold mel into pre_w, interleave channels, pack columns ----
    wrows = np.zeros((n_wcol * P, O), dtype=np.float32)
    for k in range(K):
        n = int(lengths[k])
        w2 = mel_w[k, None, :n, None] * pre_w[k, :, :n, :]  # (C, n, O)
        stacked = w2.transpose(1, 0, 2).reshape(2 * n, O)   # rows (w, c)
        off = 0
        for (c, a, p0, e, wcol) in pieces[k]:
            nreal = e - p0
            wrows[wcol * P + p0 : wcol * P + e] = stacked[off : off + nreal]
            off += nreal
    wg = np.ascontiguousarray(
        wrows.reshape(n_wcol, P, O).transpose(1, 0, 2).reshape(P, n_wcol * O)
    ).astype(_IN_NP)

    btT = np.ascontiguousarray(pre_b.T)  # (O, K) fp32

    # ---- per-core x: channel-interleaved rows (2f+c), partition-major ----
    in_maps = []
    pad_rows = n_xch * P - 2 * F
    for ci in range(N_CORES):
        sl = x[:, :, ci * T_loc : (ci + 1) * T_loc, :]  # (B, C, T_loc, F)
        xt3 = np.ascontiguousarray(sl.transpose(3, 1, 0, 2)).reshape(2 * F, BT)
        if pad_rows:
            xt3 = np.concatenate([xt3, np.zeros((pad_rows, BT), np.float32)], axis=0)
        xgc = np.ascontiguousarray(
            xt3.reshape(n_xch, P, BT).transpose(1, 0, 2).reshape(P, n_xch * BT)
        ).astype(_IN_NP)
        in_maps.append({"xg": xgc, "wg": wg, "bt": btT})

    trace = bool(os.environ.get("BANDSPLIT_TRACE"))
    if trace:
        trace = _install_trace_hook()
    res = bass_utils.run_bass_kernel_spmd(
        nc, in_maps, list(range(N_CORES)), trace=trace
    )
    LAST_RESULTS = res

    outs = np.stack([res.results[ci]["out"] for ci in range(N_CORES)], axis=0)
    # (n_cores, O, K*BT) -> (n_cores, O, K, B, T_loc) -> (B, O, T, K)
    outs = outs.reshape(N_CORES, O, K, B, T_loc)
    full = outs.transpose(3, 1, 0, 4, 2).reshape(B, O, T, K)
    return np.ascontiguousarray(full)



# revision 2
# speedup vs baseline: 1.2635x; 1.2635x over previous
"""Trainium2 Bass kernel for nn_BandSplit (v3).

Computes, for each of K mel bands:
    out[b, o, t, k] = sum_{c,w} x[b, c, t, idx[k,w]] * mel_w[k,w] * pre_w[k,c,w,o] + pre_b[k,o]

Structure exploited:
  - Band indices idx[k, :n_k] are contiguous runs (triangular mel filters),
    so the gather is a strided slice.
  - mel_w folds into pre_w on the host: W2[k,c,w,o] = mel_w[k,w]*pre_w[k,c,w,o].
  - x rows are channel-interleaved (row = 2f + c); band k's contraction is
    the contiguous row run [2s_k, 2s_k+2n_k).

v3 changes vs v2 (the 69-78us baseline):
  - The binding constraint in v2 was total DMA bytes through the slowest
    of the 16 DMA engines (engine 15 ran ~97% busy end to end).  Output
    moves to bf16 (rel-err gate is 2e-2; bf16 rounding adds ~1e-3): halves
    the dominant output stream (16.8 -> 8.4 MB/core).  Host upcasts.
  - x chunks are packed with bounded row duplication: when a band would
    cross a 128-row chunk boundary and the duplicated prefix is small, the
    chunk restarts at the band's first row instead of splitting the band
    into two matmuls.  94 -> ~86 matmuls, and weight zero-extension rows
    (split pieces force tile_position base 0) shrink.
  - The entire per-core output (64 KB/partition in bf16) is staged in SBUF
    (stage pool bufs = n_groups), so PE/DVE/ACT never block on the output
    stream; out-DMAs drain at the DMA engines' own pace.
  - Out-DMAs are one per 8 bands (8 KB/partition descriptors).
  - Warm-up memset runs on GpSimd (Pool) instead of DVE so the first
    PE warm-up matmul issues earlier; the HAM clock gate needs ~5us of
    sustained matmul activity before it lifts the PE 1.2 -> 2.4 GHz.

Sharding: data-parallel over T across 8 cores (T=1024 -> 128/core); identical
SPMD program per core, weights replicated, host reassembles (B, O, T, K).
"""

import os
import sys
import types

import numpy as np

for _p in ("/opt/trn_rl_repo",):
    if _p not in sys.path:
        sys.path.insert(0, _p)

import ml_dtypes

import concourse.bass as bass
import concourse.mybir as mybir
import concourse.tile as tile
from concourse import bass_utils

N_CORES = 8
O = 128          # out channels (= stationary free dim = PSUM partitions)
GROUP = 8        # bands per stage tile / output DMA
P = 128          # SBUF partitions / chunk rows
BT = 512         # B * T_loc columns per core
N_WARMUP = int(os.environ.get("BANDSPLIT_WARMUP", "10"))
DUP_MAX = int(os.environ.get("BANDSPLIT_DUPMAX", "32"))

_F32 = mybir.dt.float32
_IN_DT = mybir.dt.bfloat16
_IN_NP = ml_dtypes.bfloat16

if os.environ.get("BANDSPLIT_OUT_DT", "bf16") == "f32":
    _OUT_DT = mybir.dt.float32
    _OUT_NP = np.float32
else:
    _OUT_DT = mybir.dt.bfloat16
    _OUT_NP = ml_dtypes.bfloat16


# ---------------------------------------------------------------------------
# Workaround: this container's walrus rejects instructions carrying more than
# a couple of sem waits ("Too many sync wait commands", CoreV3GenImpl
# setupSyncWait).  Post-pass: move excess waits onto single-wait NoOps
# inserted just before the instruction on the same engine/sequencer.
# ---------------------------------------------------------------------------
_MAX_WAITS = 1

if os.environ.get("BANDSPLIT_LDWOPT"):
    # Experiment: let walrus overlap LDWEIGHTS with matmuls (the default
    # pipeline pins --enable-ldw-opt=false).
    _orig_run_command = bass_utils.run_command

    def _patched_run_command(cmd, **kw):
        if isinstance(cmd, list):
            cmd = [
                "--enable-ldw-opt=true" if c == "--enable-ldw-opt=false" else c
                for c in cmd
            ]
        return _orig_run_command(cmd, **kw)

    bass_utils.run_command = _patched_run_command


def _split_excess_waits(nc, max_waits=_MAX_WAITS):
    ctr = 0
    for f in nc.m.functions:
        for bb in f.blocks:
            il = bb.instructions
            i = 0
            while i < len(il):
                inst = il[i]
                si = inst.sync_info
                if si is not None and si.on_wait and len(si.on_wait) > max_waits:
                    waits = list(si.on_wait)
                    keep = waits[-max_waits:]
                    extra = waits[:-max_waits]
                    nops = []
                    for w in extra:
                        ctr += 1
                        nop = mybir.InstNoOp(
                            name=f"{inst.name}-wsplit{ctr}",
                            engine=inst.engine,
                            sync_info=mybir.SyncInfo(on_wait=[w], on_update=[]),
                            bass_nofuse=True,
                        )
                        nc.register_instruction(nop, overwrite=True)
                        nops.append(nop)
                    inst.sync_info = mybir.SyncInfo(
                        on_wait=keep, on_update=list(si.on_update or [])
                    )
                    il[i:i] = nops
                    i += len(nops)
                i += 1
    return ctr


# ---------------------------------------------------------------------------
# Optional NTFF profiling (test.py sets BANDSPLIT_TRACE=1).  The agent image's
# antenv lacks axon_hooks, so tracing degrades silently unless we install the
# ctypes-based hook ourselves.
# ---------------------------------------------------------------------------
def _install_trace_hook():
    try:
        import antenv  # noqa: F401
        from trn_agent_boot.trn_boot import _ntff_profile_via_ctypes

        if "antenv.axon_hooks" in sys.modules:
            return True
        hook = _ntff_profile_via_ctypes("/opt/axon/libaxon_pjrt.so")
        mod = types.ModuleType("antenv.axon_hooks")
        mod._hook = hook
        mod.get_axon_ntff_profile_hook = lambda: mod._hook
        mod.set_axon_ntff_profile_hook = lambda h: setattr(mod, "_hook", h)
        sys.modules["antenv.axon_hooks"] = mod
        import antenv as _ae

        _ae.axon_hooks = mod
        return True
    except Exception:
        return False


# ---------------------------------------------------------------------------
# Band structure extraction (host side, from the actual inputs)
# ---------------------------------------------------------------------------
def _band_structure(idx, mel_w):
    idx = np.asarray(idx)
    mel_w = np.asarray(mel_w)
    K = idx.shape[0]
    starts = np.empty(K, dtype=np.int64)
    lengths = np.empty(K, dtype=np.int64)
    for k in range(K):
        nz = np.nonzero(mel_w[k])[0]
        assert nz.size > 0, f"band {k} empty"
        n = int(nz.max()) + 1
        run = idx[k, :n]
        assert np.all(np.diff(run) == 1), f"band {k} indices not contiguous"
        starts[k] = int(run[0])
        lengths[k] = n
    return starts, lengths


def _align_base(p0, e):
    """Largest legal 32-aligned base <= p0 for a piece ending at e.

    tile_position rule: rows<=32 -> base in {0,32,64,96}; rows<=64 -> {0,64};
    rows>64 -> base 0.
    """
    for a in (96, 64, 32, 0):
        if a > p0:
            continue
        rows = e - a
        if rows <= 32 or (rows <= 64 and a in (0, 64)) or a == 0:
            return a
    raise AssertionError((p0, e))


# HW note: nonzero tile_position row bases are only safe for single-matmul
# bands (start=stop=True).  Mixing bases inside a PSUM accumulation group
# (split bands) aborts the NEFF at runtime on this stack, so split bands'
# pieces all use base 0 (with zero weight rows below p0).


def _plan(starts, lengths, F):
    """Pack band row-runs into 128-row chunks with bounded duplication.

    Chunks are arbitrary 128-row windows of the channel-interleaved row
    space, created in increasing start order.  A band crossing the current
    window restarts a fresh window at its own first row when the duplicated
    prefix is <= DUP_MAX rows (one matmul instead of two); otherwise it
    splits along the natural continuation grid.

    Returns:
      chunk_rows  -> list of chunk start rows (virtual row space, 2F wide)
      pieces[k]   -> list of (chunk, base, p0, e, wcol); rows [base,p0) are
                     zero weight extension, [p0,e) real
      n_wcol      -> number of packed 128-row weight columns
      wcol_first  -> first band using each weight column (ascending)
    """
    K = len(starts)
    chunks = []
    raw = []
    for k in range(K):
        a = 2 * int(starts[k])
        b = a + 2 * int(lengths[k])
        pl = []
        ci = None
        for i in range(len(chunks) - 1, -1, -1):
            if chunks[i] <= a:
                ci = i
                break
        if ci is None or a >= chunks[ci] + P:
            chunks.append(a)
            ci = len(chunks) - 1
        if b <= chunks[ci] + P:
            pl.append((ci, a - chunks[ci], b - a))
        else:
            dup = chunks[ci] + P - a
            if b - a <= P and dup <= DUP_MAX:
                chunks.append(a)
                ci = len(chunks) - 1
                pl.append((ci, 0, b - a))
            else:
                r = a
                while r < b:
                    if r >= chunks[ci] + P:
                        if ci + 1 < len(chunks) and chunks[ci + 1] <= r:
                            ci += 1
                        else:
                            chunks.append(chunks[ci] + P)
                            ci = len(chunks) - 1
                    e = min(b, chunks[ci] + P)
                    pl.append((ci, r - chunks[ci], e - r))
                    r = e
        raw.append(pl)

    # weight column packing: first-fit on 32-row granules, in band order
    # (columns are created in first-use order, so column index ranges can be
    # streamed in consumption order; backfills only ever land in columns
    # that load earlier than needed).
    pieces = [[] for _ in range(K)]
    col_fill = []
    wcol_first = []
    for k in range(K):
        single = len(raw[k]) == 1
        for (c, p0, rows) in raw[k]:
            e = p0 + rows
            a0 = _align_base(p0, e) if single else 0
            s_lo, s_hi = a0 // 32, (e + 31) // 32
            wcol = None
            for j in range(len(col_fill)):
                if col_fill[j] <= s_lo:
                    wcol = j
                    col_fill[j] = s_hi
                    break
            if wcol is None:
                col_fill.append(s_hi)
                wcol_first.append(k)
                wcol = len(col_fill) - 1
            pieces[k].append((c, a0, p0, e, wcol))
    return chunks, pieces, len(col_fill), wcol_first


# ---------------------------------------------------------------------------
# Device program
# ---------------------------------------------------------------------------
def _build_program(chunk_rows, pieces, n_wcol, wcol_first, K, with_bias):
    nc = bass.Bass("TRN2", target_bir_lowering=False, debug=False)
    n_xch = len(chunk_rows)
    xg = nc.dram_tensor("xg", [P, n_xch * BT], _IN_DT, kind="ExternalInput").ap()
    wg = nc.dram_tensor("wg", [P, n_wcol * O], _IN_DT, kind="ExternalInput").ap()
    if with_bias:
        bt = nc.dram_tensor("bt", [O, K], _F32, kind="ExternalInput").ap()
    out = nc.dram_tensor("out", [O, K * BT], _OUT_DT, kind="ExternalOutput").ap()

    n_groups = K // GROUP

    # input segmentation, interleaved by first-use band so each transfer
    # lands just before its consumers.  ~3 x segments, ~4 w segments.
    ch_first = [None] * n_xch
    for k in range(K):
        for (c, _, _, _, _) in pieces[k]:
            if ch_first[c] is None:
                ch_first[c] = k
    for c in range(n_xch):
        if ch_first[c] is None:
            ch_first[c] = K
    n_xseg = min(3, n_xch)
    xsegs = []  # (first_band, clo, chi)
    per = (n_xch + n_xseg - 1) // n_xseg
    for s in range(n_xseg):
        clo, chi = s * per, min((s + 1) * per - 1, n_xch - 1)
        if clo > chi:
            continue
        xsegs.append((min(ch_first[c] for c in range(clo, chi + 1)), clo, chi))
    n_wseg = min(4, n_wcol)
    wsegs = []  # (first_band, wlo, whi)
    per = (n_wcol + n_wseg - 1) // n_wseg
    for s in range(n_wseg):
        wlo, whi = s * per, min((s + 1) * per - 1, n_wcol - 1)
        if wlo > whi:
            continue
        wsegs.append((min(wcol_first[j] for j in range(wlo, whi + 1)), wlo, whi))
    loads = sorted(
        [("x", i, fb) for i, (fb, _, _) in enumerate(xsegs)]
        + [("w", i, fb) for i, (fb, _, _) in enumerate(wsegs)],
        key=lambda t: (t[2], t[0] == "x"),
    )

    import contextlib

    with tile.TileContext(nc) as tc:
        with contextlib.ExitStack() as ctx:
            stage_pool = ctx.enter_context(
                tc.tile_pool(name="stage", bufs=n_groups)
            )
            psum_pool = ctx.enter_context(
                tc.tile_pool(name="psum", bufs=4, space="PSUM")
            )
            warm_pool = ctx.enter_context(tc.tile_pool(name="warm", bufs=1))
            if with_bias:
                bias_pool = ctx.enter_context(tc.tile_pool(name="bias", bufs=1))

            # --- PE warm-up: the HAM clock gate keeps the PE at 1.2 GHz
            # until it has seen ~5us of sustained matmul activity.  Burn
            # dummy matmuls during the DMA preload so the clock lifts to
            # 2.4 GHz around the time the real stream starts.  Memset on
            # GpSimd so the first warm-up matmul issues as early as
            # possible.
            if N_WARMUP:
                wdum = warm_pool.tile([P, O + BT], _IN_DT)
                nc.gpsimd.memset(wdum[:, :], 0)
                for _ in range(N_WARMUP):
                    pw = psum_pool.tile([O, 2 * BT], _F32, tag="psum")
                    nc.tensor.matmul(
                        pw[:, :BT],
                        wdum[:, :O],
                        wdum[:, O : O + BT],
                        start=True,
                        stop=True,
                        tile_position=(0, 0),
                    )

            if with_bias:
                bias_t = bias_pool.tile([O, K], _F32)
                nc.sync.dma_start(out=bias_t[:, :], in_=bt[:, :])

            # input loads on the Sync (HWDGE) ring, in consumption order
            xtiles = [None] * len(xsegs)
            wtiles = [None] * len(wsegs)
            for (kind, i, _) in loads:
                if kind == "x":
                    _, clo, chi = xsegs[i]
                    xp = ctx.enter_context(tc.tile_pool(name=f"xseg{i}", bufs=1))
                    xt = xp.tile([P, (chi - clo + 1) * BT], _IN_DT)
                    nc.sync.dma_start(
                        out=xt[:, :], in_=xg[:, clo * BT : (chi + 1) * BT]
                    )
                    xtiles[i] = (xt, clo, chi)
                else:
                    _, wlo, whi = wsegs[i]
                    wp = ctx.enter_context(tc.tile_pool(name=f"wseg{i}", bufs=1))
                    wt = wp.tile([P, (whi - wlo + 1) * O], _IN_DT)
                    nc.sync.dma_start(
                        out=wt[:, :], in_=wg[:, wlo * O : (whi + 1) * O]
                    )
                    wtiles[i] = (wt, wlo, whi)

            def xof(c):
                for (xt, clo, chi) in xtiles:
                    if clo <= c <= chi:
                        return xt, c - clo
                raise AssertionError(c)

            def wof(j):
                for (wt, wlo, whi) in wtiles:
                    if wlo <= j <= whi:
                        return wt, j - wlo
                raise AssertionError(j)

            pair_ctr = 0
            for g in range(n_groups):
                stage = stage_pool.tile([O, GROUP * BT], _OUT_DT, tag="stage")
                # Bands processed in pairs sharing a 2-bank PSUM tile: halves
                # the psum-slot semaphore checks on the PE and halves the
                # PSUM->SBUF copy op count.
                for jp in range(GROUP // 2):
                    psum = psum_pool.tile([O, 2 * BT], _F32, tag="psum")
                    for jj in range(2):
                        j = jp * 2 + jj
                        k = g * GROUP + j
                        plist = pieces[k]
                        pslice = psum[:, jj * BT : (jj + 1) * BT]
                        for pi, (c, a, p0, e, wcol) in enumerate(plist):
                            xt, lc = xof(c)
                            wt, wc = wof(wcol)
                            nc.tensor.matmul(
                                pslice,
                                wt[a:e, wc * O : (wc + 1) * O],
                                xt[a:e, lc * BT : (lc + 1) * BT],
                                start=(pi == 0),
                                stop=(pi == len(plist) - 1),
                                tile_position=(a, 0),
                            )
                    if with_bias:
                        for jj in range(2):
                            j = jp * 2 + jj
                            k = g * GROUP + j
                            nc.vector.tensor_scalar_add(
                                out=stage[:, j * BT : (j + 1) * BT],
                                in0=psum[:, jj * BT : (jj + 1) * BT],
                                scalar1=bias_t[:, k : k + 1],
                            )
                    else:
                        dst = stage[:, jp * 2 * BT : (jp * 2 + 2) * BT]
                        if pair_ctr % 2 == 0:
                            nc.vector.tensor_copy(dst, psum[:, :])
                        else:
                            nc.scalar.copy(dst, psum[:, :])
                    pair_ctr += 1
                # GpSimd/SWDGE ring: keeps outputs off the Sync ring and
                # off the compute engines.
                nc.gpsimd.dma_start(
                    out=out[:, g * GROUP * BT : (g + 1) * GROUP * BT],
                    in_=stage[:, :],
                )
    _split_excess_waits(nc)
    return nc


_CACHE = {}
LAST_RESULTS = None


def kernel(x, idx, mel_w, pre_w, pre_b):
    global LAST_RESULTS
    x = np.ascontiguousarray(np.asarray(x, dtype=np.float32))
    pre_w = np.asarray(pre_w, dtype=np.float32)
    pre_b = np.asarray(pre_b, dtype=np.float32)
    mel_w = np.asarray(mel_w, dtype=np.float32)
    B, C, T, F = x.shape
    K = np.asarray(idx).shape[0]
    assert C == 2 and T % N_CORES == 0
    T_loc = T // N_CORES
    assert B * T_loc == BT and pre_w.shape[-1] == O and K % GROUP == 0

    starts, lengths = _band_structure(idx, mel_w)
    with_bias = bool(np.any(pre_b != 0.0))
    key = (B, C, T, F, K, with_bias, starts.tobytes(), lengths.tobytes())
    if key not in _CACHE:
        chunk_rows, pieces, n_wcol, wcol_first = _plan(starts, lengths, F)
        nc = _build_program(chunk_rows, pieces, n_wcol, wcol_first, K, with_bias)
        _CACHE[key] = (nc, chunk_rows, pieces, n_wcol)
    nc, chunk_rows, pieces, n_wcol = _CACHE[key]
    n_xch = len(chunk_rows)

    # ---- weights: fold mel into pre_w, interleave channels, pack columns ----
    wrows = np.zeros((n_wcol * P, O), dtype=np.float32)
    for k in range(K):
        n = int(lengths[k])
        w2 = mel_w[k, None, :n, None] * pre_w[k, :, :n, :]  # (C, n, O)
        stacked = w2.transpose(1, 0, 2).reshape(2 * n, O)   # rows (w, c)
        off = 0
        for (c, a, p0, e, wcol) in pieces[k]:
            nreal = e - p0
            wrows[wcol * P + p0 : wcol * P + e] = stacked[off : off + nreal]
            off += nreal
    wg = np.ascontiguousarray(
        wrows.reshape(n_wcol, P, O).transpose(1, 0, 2).reshape(P, n_wcol * O)
    ).astype(_IN_NP)

    btT = np.ascontiguousarray(pre_b.T)  # (O, K) fp32

    # ---- per-core x: channel-interleaved rows (2f+c) gathered per chunk ----
    # virtual row v = 2f + c; chunk cc takes rows [chunk_rows[cc], +128)
    row_idx = np.concatenate(
        [np.arange(r0, r0 + P) for r0 in chunk_rows]
    )  # (n_xch*P,)
    valid = row_idx < 2 * F
    row_idx_c = np.where(valid, row_idx, 0)
    in_maps = []
    for ci in range(N_CORES):
        sl = x[:, :, ci * T_loc : (ci + 1) * T_loc, :]  # (B, C, T_loc, F)
        xt3 = np.ascontiguousarray(sl.transpose(3, 1, 0, 2)).reshape(2 * F, BT)
        gath = xt3[row_idx_c]
        gath[~valid] = 0.0
        xgc = np.ascontiguousarray(
            gath.reshape(n_xch, P, BT).transpose(1, 0, 2).reshape(P, n_xch * BT)
        ).astype(_IN_NP)
        m = {"xg": xgc, "wg": wg}
        if with_bias:
            m["bt"] = btT
        in_maps.append(m)

    trace = bool(os.environ.get("BANDSPLIT_TRACE"))
    if trace:
        trace = _install_trace_hook()
    res = bass_utils.run_bass_kernel_spmd(
        nc, in_maps, list(range(N_CORES)), trace=trace
    )
    LAST_RESULTS = res

    outs = np.stack(
        [np.asarray(res.results[ci]["out"], dtype=np.float32) for ci in range(N_CORES)],
        axis=0,
    )
    # (n_cores, O, K*BT) -> (n_cores, O, K, B, T_loc) -> (B, O, T, K)
    outs = outs.reshape(N_CORES, O, K, B, T_loc)
    full = outs.transpose(3, 1, 0, 4, 2).reshape(B, O, T, K)
    return np.ascontiguousarray(full)
